# revision 1
# baseline (speedup 1.0000x reference)
"""DGCN aggregation kernel for Trainium2 (8 NeuronCores, graph-parallel).

Math (per edge type t):
    xn     = (x - mu) / sigma                      (feature-wise, ddof=1)
    deg_t  = segsum(|ea_t|, dst) + 1
    S'_t[d, s] = sum_{e:(s->d)} dis[s] |ea| dis[d]   (+ 1/deg on the diagonal)
    h1_t   = relu(S'_t xn W1_t + b1_t)
    out_t  = relu(S'_t h1_t W2_t + b2_t)
    out    = concat_t(out_t) reshaped to (B*NN, S, 3*D2)

Device mapping: S' application is a gather (by src) + one-hot matmul
(segment-sum by dst) with edges sorted by dst and sharded across 8 cores by
contiguous 4096-node dst ranges.  Normalization is folded into the weights:
    S' xn W1 = (S' x) (D W1) - (S' 1) (mu D W1)    with D = diag(1/sigma)
so the device only ever gathers raw x rows; the rank-1 correction
(w = S'1 outer c = mu D W1) is one extra K=1 matmul accumulated into PSUM.
Layer 2 associates as S'(h1 W2): the per-node g = h1 W2 table (fp16) is
assembled on the host between the two launches and gathered by src.
"""

import numpy as np

import concourse.bacc as bacc
import concourse.mybir as mybir
import concourse.tile as tile
from concourse.bass_utils import run_bass_kernel_spmd

F32 = mybir.dt.float32
F16 = mybir.dt.float16
I16 = mybir.dt.int16

# Problem constants (hardcoded per the harness contract).
N = 32768          # nodes = B*S*NN = 4*16*512
E = 524288         # edges
F_IN, D1, D2 = 128, 256, 128
NT = 3             # edge types
BATCH, SEQ, NNODE = 4, 16, 512

NCORES = 8
NPC = N // NCORES          # nodes per core = 4096
GROUP = 16                 # dst nodes per one-hot group
BPG = 3                    # 128-edge batches per group (fixed padding)
SLOTS_PG = BPG * 128       # padded edge slots per group = 384
GROUPS_PC = NPC // GROUP   # 256 groups per core
BATCHES_PC = GROUPS_PC * BPG          # 768 batches per core
SLOTS_PC = GROUPS_PC * SLOTS_PG       # 98304 edge slots per core
TILES_PC = NPC // 128      # 32 dst tiles per core
BPT = BPG * 8              # batches per dst tile = 24
W_OH = NT * GROUP          # one-hot width = 48

# Set by test.py for profiling runs; grading runs keep this off.
TRACE = False
LAST_TIMING = {}

_NC_CACHE = {}


def _build_l1():
    nc = bacc.Bacc("TRN2", target_bir_lowering=False, debug=False)
    x_tab = nc.dram_tensor("x_tab", [N, F_IN], F32, kind="ExternalInput")
    idx = nc.dram_tensor("idx", [128, SLOTS_PC // 16], I16, kind="ExternalInput")
    oh = nc.dram_tensor("oh", [128, BATCHES_PC, W_OH], F32, kind="ExternalInput")
    w1p = nc.dram_tensor("w1p", [F_IN, NT, D1], F32, kind="ExternalInput")
    negc = nc.dram_tensor("negc", [1, NT * D1], F32, kind="ExternalInput")
    b1 = nc.dram_tensor("b1", [128, NT * 2], F32, kind="ExternalInput")
    w2 = nc.dram_tensor("w2", [128, NT, 2, D2], F32, kind="ExternalInput")
    wv = nc.dram_tensor("wv", [1, NT * NPC], F32, kind="ExternalInput")
    g16 = nc.dram_tensor("g16", [NPC, NT * D2], F16, kind="ExternalOutput")

    with tile.TileContext(nc) as tc:
        with (
            tc.tile_pool(name="const", bufs=1) as cpool,
            tc.tile_pool(name="sb", bufs=2) as sb,
            tc.tile_pool(name="ps", bufs=2, space="PSUM") as ps,
            tc.tile_pool(name="ps2", bufs=2, space="PSUM") as ps2,
            tc.tile_pool(name="ps3", bufs=2, space="PSUM") as ps3,
        ):
            idx_t = cpool.tile([128, SLOTS_PC // 16], I16)
            nc.sync.dma_start(out=idx_t[:], in_=idx[:, :])
            w1p_t = cpool.tile([F_IN, NT, D1], F32)
            nc.sync.dma_start(out=w1p_t[:], in_=w1p[:, :, :])
            negc_t = cpool.tile([1, NT * D1], F32)
            nc.sync.dma_start(out=negc_t[:], in_=negc[:, :])
            b1_t = cpool.tile([128, NT * 2], F32)
            nc.sync.dma_start(out=b1_t[:], in_=b1[:, :])
            w2_t = cpool.tile([128, NT, 2, D2], F32)
            nc.sync.dma_start(out=w2_t[:], in_=w2[:, :, :, :])
            wv_t = cpool.tile([1, NT * NPC], F32)
            nc.sync.dma_start(out=wv_t[:], in_=wv[:, :])

            for ti in range(TILES_PC):
                xg = sb.tile([128, BPT, F_IN], F32, tag="xg")
                nc.gpsimd.dma_gather(
                    xg[:], x_tab[:, :],
                    idx_t[:, ti * (BPT * 8):(ti + 1) * (BPT * 8)],
                    BPT * 128, BPT * 128, F_IN, single_packet=False,
                )
                oh_t = sb.tile([128, BPT, W_OH], F32, tag="oh")
                nc.sync.dma_start(
                    out=oh_t[:], in_=oh[:, ti * BPT:(ti + 1) * BPT, :])

                # m1T[f, (group, type, slot)] accumulated per 16-node group
                m1_ps = ps.tile([128, 8 * W_OH], F32, space="PSUM", tag="m1")
                for g8 in range(8):
                    for b in range(BPG):
                        bl = g8 * BPG + b
                        nc.tensor.matmul(
                            out=m1_ps[:, g8 * W_OH:(g8 + 1) * W_OH],
                            lhsT=xg[:, bl, :],
                            rhs=oh_t[:, bl, :],
                            start=(b == 0), stop=(b == BPG - 1),
                        )
                g_sb = sb.tile([128, NT * D2], F16, tag="gout")
                for t in range(NT):
                    # de-interleave type t: cols g8*48 + t*16 + s -> [128, 128]
                    m1t = sb.tile([128, 128], F32, tag="m1t")
                    src_ap = m1_ps[:].rearrange(
                        "p (g t s) -> p g t s", g=8, t=NT)[:, :, t, :]
                    nc.vector.tensor_copy(out=m1t[:], in_=src_ap)
                    for c in range(2):
                        h1_ps = ps2.tile([128, 128], F32, space="PSUM", tag="h1")
                        nc.tensor.matmul(
                            out=h1_ps[:],
                            lhsT=w1p_t[:, t, c * 128:(c + 1) * 128],
                            rhs=m1t[:],
                            start=True, stop=False,
                        )
                        nc.tensor.matmul(
                            out=h1_ps[:],
                            lhsT=negc_t[:, (t * 2 + c) * 128:(t * 2 + c + 1) * 128],
                            rhs=wv_t[:, t * NPC + ti * 128: t * NPC + (ti + 1) * 128],
                            start=False, stop=True,
                        )
                        h1t = sb.tile([128, 128], F32, tag=f"h1t{c}")
                        nc.scalar.activation(
                            out=h1t[:], in_=h1_ps[:],
                            func=mybir.ActivationFunctionType.Relu,
                            bias=b1_t[:, t * 2 + c: t * 2 + c + 1], scale=1.0,
                        )
                        if c == 0:
                            h1t0 = h1t
                    g_ps = ps3.tile([128, D2], F32, space="PSUM", tag="g")
                    nc.tensor.matmul(
                        out=g_ps[:], lhsT=h1t0[:], rhs=w2_t[:, t, 0, :],
                        start=True, stop=False,
                    )
                    nc.tensor.matmul(
                        out=g_ps[:], lhsT=h1t[:], rhs=w2_t[:, t, 1, :],
                        start=False, stop=True,
                    )
                    nc.vector.tensor_copy(
                        out=g_sb[:, t * D2:(t + 1) * D2], in_=g_ps[:])
                nc.sync.dma_start(
                    out=g16[ti * 128:(ti + 1) * 128, :], in_=g_sb[:])
    nc.compile()
    return nc


def _build_l2():
    nc = bacc.Bacc("TRN2", target_bir_lowering=False, debug=False)
    g_tab = nc.dram_tensor("g_tab", [N, NT * D2], F16, kind="ExternalInput")
    idx = nc.dram_tensor("idx", [128, SLOTS_PC // 16], I16, kind="ExternalInput")
    oh = nc.dram_tensor("oh", [128, BATCHES_PC, W_OH], F16, kind="ExternalInput")
    b2 = nc.dram_tensor("b2", [128, NT], F32, kind="ExternalInput")
    out2 = nc.dram_tensor("out2", [NT, D2, NPC], F32, kind="ExternalOutput")

    with tile.TileContext(nc) as tc:
        with (
            tc.tile_pool(name="const", bufs=1) as cpool,
            tc.tile_pool(name="sb", bufs=2) as sb,
            tc.tile_pool(name="ps", bufs=2, space="PSUM") as ps,
        ):
            idx_t = cpool.tile([128, SLOTS_PC // 16], I16)
            nc.sync.dma_start(out=idx_t[:], in_=idx[:, :])
            b2_t = cpool.tile([128, NT], F32)
            nc.sync.dma_start(out=b2_t[:], in_=b2[:, :])

            for ti in range(TILES_PC):
                gg = sb.tile([128, BPT, NT * D2], F16, tag="gg")
                nc.gpsimd.dma_gather(
                    gg[:], g_tab[:, :],
                    idx_t[:, ti * (BPT * 8):(ti + 1) * (BPT * 8)],
                    BPT * 128, BPT * 128, NT * D2, single_packet=False,
                )
                oh_t = sb.tile([128, BPT, W_OH], F16, tag="oh")
                nc.sync.dma_start(
                    out=oh_t[:], in_=oh[:, ti * BPT:(ti + 1) * BPT, :])
                for t in range(NT):
                    # m2T_t [d2, node-within-tile], 16-col windows per group
                    m2_ps = ps.tile([128, 128], F32, space="PSUM", tag="m2")
                    for g8 in range(8):
                        for b in range(BPG):
                            bl = g8 * BPG + b
                            nc.tensor.matmul(
                                out=m2_ps[:, g8 * GROUP:(g8 + 1) * GROUP],
                                lhsT=gg[:, bl, t * D2:(t + 1) * D2],
                                rhs=oh_t[:, bl, t * GROUP:(t + 1) * GROUP],
                                start=(b == 0), stop=(b == BPG - 1),
                            )
                    o_sb = sb.tile([128, 128], F32, tag="osb")
                    nc.scalar.activation(
                        out=o_sb[:], in_=m2_ps[:],
                        func=mybir.ActivationFunctionType.Relu,
                        bias=b2_t[:, t:t + 1], scale=1.0,
                    )
                    nc.sync.dma_start(
                        out=out2[t, :, ti * 128:(ti + 1) * 128], in_=o_sb[:])
    nc.compile()
    return nc


def _host_prep(x, edge_attr, edge_index, W1):
    """Sort/shard/pad edges; fold normalization into weights. Returns the
    per-core device inputs plus the folded weights."""
    src = np.asarray(edge_index[0], np.int64)
    dst = np.asarray(edge_index[1], np.int64)
    ew = np.abs(np.asarray(edge_attr, np.float32))          # [E, 3]

    deg = np.empty((N, NT), np.float32)
    for t in range(NT):
        deg[:, t] = np.bincount(dst, weights=ew[:, t], minlength=N)
    deg += 1.0
    dis = 1.0 / np.sqrt(deg)

    norm = dis[src] * ew * dis[dst]                          # [E, 3]
    src_all = np.concatenate([src, np.arange(N)])
    dst_all = np.concatenate([dst, np.arange(N)])
    norm_all = np.concatenate([norm, 1.0 / deg]).astype(np.float32)

    order = np.argsort(dst_all, kind="stable")
    sa = src_all[order]
    da = dst_all[order]
    na = norm_all[order]

    # w = S' 1 (row sums, incl. self term) for the rank-1 mean correction
    w = np.empty((N, NT), np.float32)
    for t in range(NT):
        w[:, t] = np.bincount(da, weights=na[:, t], minlength=N)

    gid = da >> 4                                            # 16-node group id
    counts = np.bincount(gid, minlength=N // GROUP)
    assert counts.max() <= SLOTS_PG, (
        f"group overflow: {counts.max()} > {SLOTS_PG}")
    gstart = np.zeros(N // GROUP + 1, np.int64)
    np.cumsum(counts, out=gstart[1:])
    rank = np.arange(da.size) - gstart[gid]
    pos = gid * SLOTS_PG + rank                              # padded slot

    n_slots = (N // GROUP) * SLOTS_PG
    src_pad = np.zeros(n_slots, np.int16)
    src_pad[pos] = sa.astype(np.int16)
    oh_full = np.zeros((n_slots // 128, 128, W_OH), np.float32)
    bi = pos // 128
    pi = pos % 128
    slot = (da & (GROUP - 1)).astype(np.int64)
    for t in range(NT):
        oh_full[bi, pi, t * GROUP + slot] = na[:, t]

    # fold normalization
    mu = x.mean(axis=0)
    sg = x.std(axis=0, ddof=1)
    W1p = np.asarray(W1, np.float32) / sg[None, :, None]     # [3,128,256]
    cvec = np.einsum("f,tfd->td", mu, W1p).astype(np.float32)  # [3, 256]

    per_core = []
    for k in range(NCORES):
        s0 = k * SLOTS_PC
        idx_core = src_pad[s0:s0 + SLOTS_PC].reshape(SLOTS_PC // 16, 16).T
        idx_core = np.ascontiguousarray(np.tile(idx_core, (8, 1)))
        oh_core = np.ascontiguousarray(
            oh_full[k * BATCHES_PC:(k + 1) * BATCHES_PC].transpose(1, 0, 2))
        w_core = np.ascontiguousarray(
            w[k * NPC:(k + 1) * NPC].T.reshape(1, NT * NPC))
        per_core.append((idx_core, oh_core, w_core))
    return per_core, W1p, cvec


def kernel(x, edge_attr, W1, b1, W2, b2, edge_index, batch_size, seq_len,
           n_nodes):
    x = np.asarray(x, np.float32)
    edge_attr = np.asarray(edge_attr, np.float32)
    W1 = np.asarray(W1, np.float32)
    b1 = np.asarray(b1, np.float32)
    W2 = np.asarray(W2, np.float32)
    b2 = np.asarray(b2, np.float32)
    edge_index = np.asarray(edge_index)
    assert x.shape == (N, F_IN) and edge_index.shape == (2, E)

    per_core, W1p, cvec = _host_prep(x, edge_attr, edge_index, W1)

    # ---- launch 1 ----
    if "l1" not in _NC_CACHE:
        _NC_CACHE["l1"] = _build_l1()
    nc1 = _NC_CACHE["l1"]

    w1p_in = np.ascontiguousarray(W1p.transpose(1, 0, 2))        # [128,3,256]
    negc_in = (-cvec).reshape(1, NT * D1).astype(np.float32)     # [1, 768]
    b1_in = np.ascontiguousarray(
        b1.reshape(NT, 2, 128).transpose(2, 0, 1).reshape(128, NT * 2))
    w2_in = np.ascontiguousarray(
        W2.reshape(NT, 2, 128, D2).transpose(2, 0, 1, 3))        # [128,3,2,128]

    in_maps1 = []
    for k in range(NCORES):
        idx_core, oh_core, w_core = per_core[k]
        in_maps1.append({
            "x_tab": x, "idx": idx_core, "oh": oh_core,
            "w1p": w1p_in, "negc": negc_in, "b1": b1_in, "w2": w2_in,
            "wv": w_core,
        })
    res1 = run_bass_kernel_spmd(
        nc1, in_maps1, core_ids=list(range(NCORES)), trace=TRACE)
    if TRACE:
        LAST_TIMING["l1_ns"] = res1.exec_time_ns

    g_full = np.concatenate(
        [res1.results[k]["g16"] for k in range(NCORES)], axis=0)  # [N, 384] f16

    # ---- launch 2 ----
    if "l2" not in _NC_CACHE:
        _NC_CACHE["l2"] = _build_l2()
    nc2 = _NC_CACHE["l2"]

    b2_in = np.ascontiguousarray(b2.T)                            # [128, 3]
    in_maps2 = []
    for k in range(NCORES):
        idx_core, oh_core, _ = per_core[k]
        in_maps2.append({
            "g_tab": g_full, "idx": idx_core,
            "oh": oh_core.astype(np.float16), "b2": b2_in,
        })
    res2 = run_bass_kernel_spmd(
        nc2, in_maps2, core_ids=list(range(NCORES)), trace=TRACE)
    if TRACE:
        LAST_TIMING["l2_ns"] = res2.exec_time_ns

    m2t = np.concatenate(
        [res2.results[k]["out2"] for k in range(NCORES)], axis=2)  # [3,128,N]

    # [3, 128, (b, s, nn)] -> out[(b, nn), s, (t, d)]
    out = m2t.reshape(NT, D2, BATCH, SEQ, NNODE).transpose(2, 4, 3, 0, 1)
    out = np.ascontiguousarray(
        out.reshape(BATCH * NNODE, SEQ, NT * D2), dtype=np.float32)
    return out



# revision 2
# speedup vs baseline: 3.4885x; 3.4885x over previous
"""DGCN aggregation kernel for Trainium2 (8 NeuronCores, graph-parallel).

Math (per edge type t):
    xn     = (x - mu) / sigma                      (feature-wise, ddof=1)
    deg_t  = segsum(|ea_t|, dst) + 1
    S'_t[d, s] = sum_{e:(s->d)} dis[s] |ea| dis[d]   (+ 1/deg on the diagonal)
    h1_t   = relu(S'_t xn W1_t + b1_t)
    out_t  = relu(S'_t h1_t W2_t + b2_t)
    out    = concat_t(out_t) reshaped to (B*NN, S, 3*D2)

Device mapping: edges (+ implicit self loops) are sorted by dst and padded
into 16-dst-node groups of 384 slots; each 128-slot batch feeds a one-hot
matmul (segment-sum by dst).  Work is sharded across 8 cores by contiguous
4096-node dst ranges.  The per-slot operand rows (xn rows for layer 1 by
src; g = h1 W2 rows for layer 2 by src) are staged by the host in slot
order, so the device only ever runs sequential streaming DMA + matmuls —
no on-device gather (SWDGE descriptor generation at ~8 ns/row was the
dominant cost).  Normalization is applied to x on the host, everything on
device is fp16 with fp32 PSUM accumulation.
"""

import numpy as np

import concourse.bacc as bacc
import concourse.mybir as mybir
import concourse.tile as tile
from concourse.bass_utils import run_bass_kernel_spmd

F32 = mybir.dt.float32
F16 = mybir.dt.float16

# Problem constants (hardcoded per the harness contract).
N = 32768          # nodes = B*S*NN = 4*16*512
E = 524288         # edges
F_IN, D1, D2 = 128, 256, 128
NT = 3             # edge types
BATCH, SEQ, NNODE = 4, 16, 512

NCORES = 8
NPC = N // NCORES          # nodes per core = 4096
GROUP = 16                 # dst nodes per one-hot group
BPG = 3                    # 128-edge batches per group (fixed padding)
SLOTS_PG = BPG * 128       # padded edge slots per group = 384
GROUPS_PC = NPC // GROUP   # 256 groups per core
BATCHES_PC = GROUPS_PC * BPG          # 768 batches per core
SLOTS_PC = GROUPS_PC * SLOTS_PG       # 98304 edge slots per core
TILES_PC = NPC // 128      # 32 dst tiles per core
BPT = BPG * 8              # batches per dst tile = 24
W_OH = NT * GROUP          # one-hot width = 48

# Set by test.py for profiling runs; grading runs keep this off.
TRACE = False
LAST_TIMING = {}

_NC_CACHE = {}


def _build_l1():
    nc = bacc.Bacc("TRN2", target_bir_lowering=False, debug=False)
    xe = nc.dram_tensor("xe", [128, BATCHES_PC, F_IN], F16, kind="ExternalInput")
    oh = nc.dram_tensor("oh", [128, BATCHES_PC, W_OH], F16, kind="ExternalInput")
    w1 = nc.dram_tensor("w1", [F_IN, NT, D1], F16, kind="ExternalInput")
    b1 = nc.dram_tensor("b1", [128, NT * 2], F32, kind="ExternalInput")
    w2 = nc.dram_tensor("w2", [128, NT, 2, D2], F16, kind="ExternalInput")
    g16 = nc.dram_tensor("g16", [NPC, NT * D2], F16, kind="ExternalOutput")

    with tile.TileContext(nc) as tc:
        with (
            tc.tile_pool(name="const", bufs=1) as cpool,
            tc.tile_pool(name="sb", bufs=2) as sb,
            tc.tile_pool(name="ps", bufs=2, space="PSUM") as ps,
            tc.tile_pool(name="ps2", bufs=2, space="PSUM") as ps2,
            tc.tile_pool(name="ps3", bufs=2, space="PSUM") as ps3,
        ):
            w1_t = cpool.tile([F_IN, NT, D1], F16)
            nc.sync.dma_start(out=w1_t[:], in_=w1[:, :, :])
            b1_t = cpool.tile([128, NT * 2], F32)
            nc.sync.dma_start(out=b1_t[:], in_=b1[:, :])
            w2_t = cpool.tile([128, NT, 2, D2], F16)
            nc.sync.dma_start(out=w2_t[:], in_=w2[:, :, :, :])

            for ti in range(TILES_PC):
                xg = sb.tile([128, BPT, F_IN], F16, tag="xg")
                nc.sync.dma_start(
                    out=xg[:], in_=xe[:, ti * BPT:(ti + 1) * BPT, :])
                oh_t = sb.tile([128, BPT, W_OH], F16, tag="oh")
                nc.sync.dma_start(
                    out=oh_t[:], in_=oh[:, ti * BPT:(ti + 1) * BPT, :])

                # m1T[f, (group, type, slot)] accumulated per 16-node group
                m1_ps = ps.tile([128, 8 * W_OH], F32, space="PSUM", tag="m1")
                for g8 in range(8):
                    for b in range(BPG):
                        bl = g8 * BPG + b
                        nc.tensor.matmul(
                            out=m1_ps[:, g8 * W_OH:(g8 + 1) * W_OH],
                            lhsT=xg[:, bl, :],
                            rhs=oh_t[:, bl, :],
                            start=(b == 0), stop=(b == BPG - 1),
                        )
                g_sb = sb.tile([128, NT * D2], F16, tag="gout")
                for t in range(NT):
                    # de-interleave type t: cols g8*48 + t*16 + s -> [128, 128]
                    m1t = sb.tile([128, 128], F16, tag="m1t")
                    src_ap = m1_ps[:].rearrange(
                        "p (g t s) -> p g t s", g=8, t=NT)[:, :, t, :]
                    nc.vector.tensor_copy(out=m1t[:], in_=src_ap)
                    for c in range(2):
                        h1_ps = ps2.tile([128, 128], F32, space="PSUM", tag="h1")
                        nc.tensor.matmul(
                            out=h1_ps[:],
                            lhsT=w1_t[:, t, c * 128:(c + 1) * 128],
                            rhs=m1t[:],
                            start=True, stop=True,
                        )
                        h1t = sb.tile([128, 128], F16, tag=f"h1t{c}")
                        nc.scalar.activation(
                            out=h1t[:], in_=h1_ps[:],
                            func=mybir.ActivationFunctionType.Relu,
                            bias=b1_t[:, t * 2 + c: t * 2 + c + 1], scale=1.0,
                        )
                        if c == 0:
                            h1t0 = h1t
                    g_ps = ps3.tile([128, D2], F32, space="PSUM", tag="g")
                    nc.tensor.matmul(
                        out=g_ps[:], lhsT=h1t0[:], rhs=w2_t[:, t, 0, :],
                        start=True, stop=False,
                    )
                    nc.tensor.matmul(
                        out=g_ps[:], lhsT=h1t[:], rhs=w2_t[:, t, 1, :],
                        start=False, stop=True,
                    )
                    nc.vector.tensor_copy(
                        out=g_sb[:, t * D2:(t + 1) * D2], in_=g_ps[:])
                nc.sync.dma_start(
                    out=g16[ti * 128:(ti + 1) * 128, :], in_=g_sb[:])
    nc.compile()
    return nc


def _build_l2():
    nc = bacc.Bacc("TRN2", target_bir_lowering=False, debug=False)
    ge = nc.dram_tensor("ge", [SLOTS_PC, NT * D2], F16, kind="ExternalInput")
    oh = nc.dram_tensor("oh", [128, BATCHES_PC, W_OH], F16, kind="ExternalInput")
    b2 = nc.dram_tensor("b2", [128, NT], F32, kind="ExternalInput")
    out2 = nc.dram_tensor("out2", [NT, D2, NPC], F16, kind="ExternalOutput")

    # slot s lives at ge[(b*128 + p), :] with b = s//128, p = s%128
    ge_pbf = ge[:].rearrange("(b p) f -> p b f", p=128)

    with tile.TileContext(nc) as tc:
        with (
            tc.tile_pool(name="const", bufs=1) as cpool,
            tc.tile_pool(name="sb", bufs=2) as sb,
            tc.tile_pool(name="ps", bufs=2, space="PSUM") as ps,
        ):
            b2_t = cpool.tile([128, NT], F32)
            nc.sync.dma_start(out=b2_t[:], in_=b2[:, :])

            for ti in range(TILES_PC):
                gg = sb.tile([128, BPT, NT * D2], F16, tag="gg")
                nc.sync.dma_start(
                    out=gg[:], in_=ge_pbf[:, ti * BPT:(ti + 1) * BPT, :])
                oh_t = sb.tile([128, BPT, W_OH], F16, tag="oh")
                nc.sync.dma_start(
                    out=oh_t[:], in_=oh[:, ti * BPT:(ti + 1) * BPT, :])
                for t in range(NT):
                    # m2T_t [d2, node-within-tile], 16-col windows per group
                    m2_ps = ps.tile([128, 128], F32, space="PSUM", tag="m2")
                    for g8 in range(8):
                        for b in range(BPG):
                            bl = g8 * BPG + b
                            nc.tensor.matmul(
                                out=m2_ps[:, g8 * GROUP:(g8 + 1) * GROUP],
                                lhsT=gg[:, bl, t * D2:(t + 1) * D2],
                                rhs=oh_t[:, bl, t * GROUP:(t + 1) * GROUP],
                                start=(b == 0), stop=(b == BPG - 1),
                            )
                    o_sb = sb.tile([128, 128], F16, tag="osb")
                    nc.scalar.activation(
                        out=o_sb[:], in_=m2_ps[:],
                        func=mybir.ActivationFunctionType.Relu,
                        bias=b2_t[:, t:t + 1], scale=1.0,
                    )
                    nc.sync.dma_start(
                        out=out2[t, :, ti * 128:(ti + 1) * 128], in_=o_sb[:])
    nc.compile()
    return nc


def _host_prep(x, edge_attr, edge_index):
    """Sort/shard/pad edges, normalize x, and stage the layer-1 per-slot
    operand stream.  Returns (xn16, per-core slot indices, per-core xe
    streams, per-core one-hot blocks)."""
    src = np.asarray(edge_index[0], np.int64)
    dst = np.asarray(edge_index[1], np.int64)
    ew = np.abs(np.asarray(edge_attr, np.float32))          # [E, 3]

    deg = np.empty((N, NT), np.float32)
    for t in range(NT):
        deg[:, t] = np.bincount(dst, weights=ew[:, t], minlength=N)
    deg += 1.0
    dis = 1.0 / np.sqrt(deg)

    norm = dis[src] * ew * dis[dst]                          # [E, 3]
    src_all = np.concatenate([src, np.arange(N)])
    dst_all = np.concatenate([dst, np.arange(N)])
    norm_all = np.concatenate([norm, 1.0 / deg]).astype(np.float32)

    order = np.argsort(dst_all, kind="stable")
    sa = src_all[order]
    da = dst_all[order]
    na = norm_all[order]

    gid = da >> 4                                            # 16-node group id
    counts = np.bincount(gid, minlength=N // GROUP)
    assert counts.max() <= SLOTS_PG, (
        f"group overflow: {counts.max()} > {SLOTS_PG}")
    gstart = np.zeros(N // GROUP + 1, np.int64)
    np.cumsum(counts, out=gstart[1:])
    rank = np.arange(da.size) - gstart[gid]
    pos = gid * SLOTS_PG + rank                              # padded slot

    n_slots = (N // GROUP) * SLOTS_PG
    src_pad = np.zeros(n_slots, np.int64)
    src_pad[pos] = sa
    oh_full = np.zeros((n_slots // 128, 128, W_OH), np.float16)
    bi = pos // 128
    pi = pos % 128
    slot = (da & (GROUP - 1)).astype(np.int64)
    for t in range(NT):
        oh_full[bi, pi, t * GROUP + slot] = na[:, t]

    # normalize x on the host; fp16 table feeds both the slot stream and
    # (via g) nothing else — device math is fp16 with fp32 accumulation
    mu = x.mean(axis=0)
    sg = x.std(axis=0, ddof=1)
    xn16 = ((x - mu[None, :]) / sg[None, :]).astype(np.float16)

    per_core = []
    for k in range(NCORES):
        s0 = k * SLOTS_PC
        idx_core = src_pad[s0:s0 + SLOTS_PC]
        # xe[p, b, :] = xn[src_pad[b*128 + p]] — one-hot batches use
        # partition = slot % 128, batch = slot // 128
        xe_core = np.ascontiguousarray(
            xn16.take(idx_core.reshape(BATCHES_PC, 128).T, axis=0))
        oh_core = np.ascontiguousarray(
            oh_full[k * BATCHES_PC:(k + 1) * BATCHES_PC].transpose(1, 0, 2))
        per_core.append((idx_core, xe_core, oh_core))
    return per_core


def kernel(x, edge_attr, W1, b1, W2, b2, edge_index, batch_size, seq_len,
           n_nodes):
    x = np.asarray(x, np.float32)
    edge_attr = np.asarray(edge_attr, np.float32)
    W1 = np.asarray(W1, np.float32)
    b1 = np.asarray(b1, np.float32)
    W2 = np.asarray(W2, np.float32)
    b2 = np.asarray(b2, np.float32)
    edge_index = np.asarray(edge_index)
    assert x.shape == (N, F_IN) and edge_index.shape == (2, E)

    per_core = _host_prep(x, edge_attr, edge_index)

    # ---- launch 1 ----
    if "l1" not in _NC_CACHE:
        _NC_CACHE["l1"] = _build_l1()
    nc1 = _NC_CACHE["l1"]

    w1_in = np.ascontiguousarray(W1.transpose(1, 0, 2)).astype(np.float16)
    b1_in = np.ascontiguousarray(
        b1.reshape(NT, 2, 128).transpose(2, 0, 1).reshape(128, NT * 2))
    w2_in = np.ascontiguousarray(
        W2.reshape(NT, 2, 128, D2).transpose(2, 0, 1, 3)).astype(np.float16)

    in_maps1 = []
    for k in range(NCORES):
        _, xe_core, oh_core = per_core[k]
        in_maps1.append({
            "xe": xe_core, "oh": oh_core,
            "w1": w1_in, "b1": b1_in, "w2": w2_in,
        })
    res1 = run_bass_kernel_spmd(
        nc1, in_maps1, core_ids=list(range(NCORES)), trace=TRACE)
    if TRACE:
        LAST_TIMING["l1_ns"] = res1.exec_time_ns

    g_full = np.concatenate(
        [res1.results[k]["g16"] for k in range(NCORES)], axis=0)  # [N, 384] f16

    # ---- launch 2 ----
    if "l2" not in _NC_CACHE:
        _NC_CACHE["l2"] = _build_l2()
    nc2 = _NC_CACHE["l2"]

    b2_in = np.ascontiguousarray(b2.T)                            # [128, 3]
    in_maps2 = []
    for k in range(NCORES):
        idx_core, _, oh_core = per_core[k]
        in_maps2.append({
            "ge": g_full.take(idx_core, axis=0),                  # [SLOTS, 384]
            "oh": oh_core, "b2": b2_in,
        })
    res2 = run_bass_kernel_spmd(
        nc2, in_maps2, core_ids=list(range(NCORES)), trace=TRACE)
    if TRACE:
        LAST_TIMING["l2_ns"] = res2.exec_time_ns

    m2t = np.concatenate(
        [res2.results[k]["out2"] for k in range(NCORES)], axis=2)  # [3,128,N] f16

    # [3, 128, (b, s, nn)] -> out[(b, nn), s, (t, d)]
    out = m2t.astype(np.float32).reshape(NT, D2, BATCH, SEQ, NNODE)
    out = out.transpose(2, 4, 3, 0, 1)
    out = np.ascontiguousarray(
        out.reshape(BATCH * NNODE, SEQ, NT * D2), dtype=np.float32)
    return out


# revision 7
# speedup vs baseline: 4.4431x; 1.2736x over previous
"""DGCN aggregation kernel for Trainium2 (8 NeuronCores, graph-parallel).

Math (per edge type t):
    xn     = (x - mu) / sigma                      (feature-wise, ddof=1)
    deg_t  = segsum(|ea_t|, dst) + 1
    S'_t[d, s] = sum_{e:(s->d)} dis[s] |ea| dis[d]   (+ 1/deg on the diagonal)
    h1_t   = relu(S'_t xn W1_t + b1_t)
    out_t  = relu(S'_t h1_t W2_t + b2_t)
    out    = concat_t(out_t) reshaped to (B*NN, S, 3*D2)

Device mapping: edges (+ implicit self loops) are sorted by dst and padded
into 16-dst-node groups of 384 slots; each 128-slot batch feeds a one-hot
matmul (segment-sum by dst).  Work is sharded across 8 cores by contiguous
4096-node dst ranges.  The per-slot operand rows (xn rows for layer 1 by
src; g = h1 W2 rows for layer 2 by src) are staged by the host in slot
order, so the device only ever runs sequential streaming DMA + matmuls —
no on-device gather (SWDGE descriptor generation at ~8 ns/row was the
dominant cost).  Normalization is applied to x on the host, everything on
device is fp16 with fp32 PSUM accumulation.
"""

import numpy as np

import concourse.bacc as bacc
import concourse.mybir as mybir
import concourse.tile as tile
from concourse.bass_utils import run_bass_kernel_spmd

F32 = mybir.dt.float32
F16 = mybir.dt.float16

# Problem constants (hardcoded per the harness contract).
N = 32768          # nodes = B*S*NN = 4*16*512
E = 524288         # edges
F_IN, D1, D2 = 128, 256, 128
NT = 3             # edge types
BATCH, SEQ, NNODE = 4, 16, 512

NCORES = 8
NPC = N // NCORES          # nodes per core = 4096
GROUP = 16                 # dst nodes per one-hot group
BPG = 3                    # 128-edge batches per group (fixed padding)
SLOTS_PG = BPG * 128       # padded edge slots per group = 384
GROUPS_PC = NPC // GROUP   # 256 groups per core
BATCHES_PC = GROUPS_PC * BPG          # 768 batches per core
SLOTS_PC = GROUPS_PC * SLOTS_PG       # 98304 edge slots per core
TILES_PC = NPC // 128      # 32 dst tiles per core
BPT = BPG * 8              # batches per dst tile = 24
W_OH = NT * GROUP          # one-hot width = 48

# Set by test.py for profiling runs; grading runs keep this off.
TRACE = False
LAST_TIMING = {}

_NC_CACHE = {}


def _build_l1():
    nc = bacc.Bacc("TRN2", target_bir_lowering=False, debug=False)
    # per-slot stream: [xn row (128) | one-hot row (48)] packed per batch
    xeoh = nc.dram_tensor(
        "xeoh", [128, BATCHES_PC, F_IN + W_OH], F16, kind="ExternalInput")
    w1 = nc.dram_tensor("w1", [F_IN, NT, D1], F16, kind="ExternalInput")
    b1 = nc.dram_tensor("b1", [128, NT * 2], F32, kind="ExternalInput")
    w2 = nc.dram_tensor("w2", [128, NT, 2, D2], F16, kind="ExternalInput")
    g16 = nc.dram_tensor("g16", [NPC, NT * D2], F16, kind="ExternalOutput")

    with tile.TileContext(nc) as tc:
        with (
            tc.tile_pool(name="const", bufs=1) as cpool,
            tc.tile_pool(name="sb", bufs=3) as sb,
            tc.tile_pool(name="sbo", bufs=2) as sbo,
            tc.tile_pool(name="ps", bufs=2, space="PSUM") as ps,
            tc.tile_pool(name="ps2", bufs=2, space="PSUM") as ps2,
            tc.tile_pool(name="ps3", bufs=2, space="PSUM") as ps3,
        ):
            w1_t = cpool.tile([F_IN, NT, D1], F16)
            nc.sync.dma_start(out=w1_t[:], in_=w1[:, :, :])
            b1_t = cpool.tile([128, NT * 2], F32)
            nc.sync.dma_start(out=b1_t[:], in_=b1[:, :])
            w2_t = cpool.tile([128, NT, 2, D2], F16)
            nc.sync.dma_start(out=w2_t[:], in_=w2[:, :, :, :])

            for ti in range(TILES_PC):
                xg = sb.tile([128, BPT, F_IN + W_OH], F16, tag="xg")
                nc.sync.dma_start(
                    out=xg[:], in_=xeoh[:, ti * BPT:(ti + 1) * BPT, :])

                # m1T[f, (group, type, slot)] accumulated per 16-node group
                m1_ps = ps.tile([128, 8 * W_OH], F32, space="PSUM", tag="m1")
                for g8 in range(8):
                    for b in range(BPG):
                        bl = g8 * BPG + b
                        nc.tensor.matmul(
                            out=m1_ps[:, g8 * W_OH:(g8 + 1) * W_OH],
                            lhsT=xg[:, bl, :F_IN],
                            rhs=xg[:, bl, F_IN:],
                            start=(b == 0), stop=(b == BPG - 1),
                        )
                g_sb = sbo.tile([128, NT * D2], F16, tag="gout")
                for t in range(NT):
                    # de-interleave type t: cols g8*48 + t*16 + s -> [128, 128]
                    m1t = sb.tile([128, 128], F16, tag="m1t")
                    src_ap = m1_ps[:].rearrange(
                        "p (g t s) -> p g t s", g=8, t=NT)[:, :, t, :]
                    nc.vector.tensor_copy(out=m1t[:], in_=src_ap)
                    for c in range(2):
                        h1_ps = ps2.tile([128, 128], F32, space="PSUM", tag="h1")
                        nc.tensor.matmul(
                            out=h1_ps[:],
                            lhsT=w1_t[:, t, c * 128:(c + 1) * 128],
                            rhs=m1t[:],
                            start=True, stop=True,
                        )
                        h1t = sb.tile([128, 128], F16, tag=f"h1t{c}")
                        nc.scalar.activation(
                            out=h1t[:], in_=h1_ps[:],
                            func=mybir.ActivationFunctionType.Relu,
                            bias=b1_t[:, t * 2 + c: t * 2 + c + 1], scale=1.0,
                        )
                        if c == 0:
                            h1t0 = h1t
                    g_ps = ps3.tile([128, D2], F32, space="PSUM", tag="g")
                    nc.tensor.matmul(
                        out=g_ps[:], lhsT=h1t0[:], rhs=w2_t[:, t, 0, :],
                        start=True, stop=False,
                    )
                    nc.tensor.matmul(
                        out=g_ps[:], lhsT=h1t[:], rhs=w2_t[:, t, 1, :],
                        start=False, stop=True,
                    )
                    nc.vector.tensor_copy(
                        out=g_sb[:, t * D2:(t + 1) * D2], in_=g_ps[:])
                nc.sync.dma_start(
                    out=g16[ti * 128:(ti + 1) * 128, :], in_=g_sb[:])
    nc.compile()
    return nc


def _build_l2():
    nc = bacc.Bacc("TRN2", target_bir_lowering=False, debug=False)
    # per-slot stream: [g row (384) | one-hot row (48)] packed per batch
    geoh = nc.dram_tensor(
        "geoh", [128, BATCHES_PC, NT * D2 + W_OH], F16, kind="ExternalInput")
    b2 = nc.dram_tensor("b2", [128, NT], F32, kind="ExternalInput")
    out2 = nc.dram_tensor(
        "out2", [D2, TILES_PC, NT, 128], F16, kind="ExternalOutput")
    GW = NT * D2

    with tile.TileContext(nc) as tc:
        with (
            tc.tile_pool(name="const", bufs=1) as cpool,
            tc.tile_pool(name="sb", bufs=3) as sb,
            tc.tile_pool(name="sbo", bufs=2) as sbo,
            tc.tile_pool(name="ps", bufs=2, space="PSUM") as ps,
        ):
            b2_t = cpool.tile([128, NT], F32)
            nc.sync.dma_start(out=b2_t[:], in_=b2[:, :])

            for ti in range(TILES_PC):
                gg = sb.tile([128, BPT, GW + W_OH], F16, tag="gg")
                nc.sync.dma_start(
                    out=gg[:], in_=geoh[:, ti * BPT:(ti + 1) * BPT, :])
                o_sb = sbo.tile([128, NT, 128], F16, tag="osb")
                for t in range(NT):
                    # m2T_t [d2, node-within-tile], 16-col windows per group
                    m2_ps = ps.tile([128, 128], F32, space="PSUM", tag="m2")
                    for g8 in range(8):
                        for b in range(BPG):
                            bl = g8 * BPG + b
                            nc.tensor.matmul(
                                out=m2_ps[:, g8 * GROUP:(g8 + 1) * GROUP],
                                lhsT=gg[:, bl, t * D2:(t + 1) * D2],
                                rhs=gg[:, bl, GW + t * GROUP:GW + (t + 1) * GROUP],
                                start=(b == 0), stop=(b == BPG - 1),
                            )
                    nc.scalar.activation(
                        out=o_sb[:, t, :], in_=m2_ps[:],
                        func=mybir.ActivationFunctionType.Relu,
                        bias=b2_t[:, t:t + 1], scale=1.0,
                    )
                nc.sync.dma_start(out=out2[:, ti, :, :], in_=o_sb[:])
    nc.compile()
    return nc


def _host_prep(x, edge_attr, edge_index):
    """Sort/shard/pad edges, normalize x, and stage the layer-1 per-slot
    operand stream.  Returns (xn16, per-core slot indices, per-core xe
    streams, per-core one-hot blocks)."""
    src = np.asarray(edge_index[0], np.int64)
    dst = np.asarray(edge_index[1], np.int64)
    ew = np.abs(np.asarray(edge_attr, np.float32))          # [E, 3]

    deg = np.empty((N, NT), np.float32)
    for t in range(NT):
        deg[:, t] = np.bincount(dst, weights=ew[:, t], minlength=N)
    deg += 1.0
    dis = 1.0 / np.sqrt(deg)

    norm = dis[src] * ew * dis[dst]                          # [E, 3]
    src_all = np.concatenate([src, np.arange(N)])
    dst_all = np.concatenate([dst, np.arange(N)])
    norm_all = np.concatenate([norm, 1.0 / deg]).astype(np.float32)

    order = np.argsort(dst_all, kind="stable")
    sa = src_all[order]
    da = dst_all[order]
    na = norm_all[order]

    gid = da >> 4                                            # 16-node group id
    counts = np.bincount(gid, minlength=N // GROUP)
    assert counts.max() <= SLOTS_PG, (
        f"group overflow: {counts.max()} > {SLOTS_PG}")
    gstart = np.zeros(N // GROUP + 1, np.int64)
    np.cumsum(counts, out=gstart[1:])
    rank = np.arange(da.size) - gstart[gid]
    pos = gid * SLOTS_PG + rank                              # padded slot

    n_slots = (N // GROUP) * SLOTS_PG
    src_pad = np.zeros(n_slots, np.int64)
    src_pad[pos] = sa
    oh_full = np.zeros((n_slots // 128, 128, W_OH), np.float16)
    bi = pos // 128
    pi = pos % 128
    slot = (da & (GROUP - 1)).astype(np.int64)
    for t in range(NT):
        oh_full[bi, pi, t * GROUP + slot] = na[:, t]

    # normalize x on the host; fp16 table feeds both the slot stream and
    # (via g) nothing else — device math is fp16 with fp32 accumulation
    mu = x.mean(axis=0)
    sg = x.std(axis=0, ddof=1)
    xn16 = ((x - mu[None, :]) / sg[None, :]).astype(np.float16)

    per_core = []
    for k in range(NCORES):
        s0 = k * SLOTS_PC
        # [p, b] index layout: partition = slot % 128, batch = slot // 128
        idx_pb = src_pad[s0:s0 + SLOTS_PC].reshape(BATCHES_PC, 128).T
        oh_core = oh_full[k * BATCHES_PC:(k + 1) * BATCHES_PC].transpose(1, 0, 2)
        xeoh = np.empty((128, BATCHES_PC, F_IN + W_OH), np.float16)
        np.take(xn16, idx_pb, axis=0, out=xeoh[:, :, :F_IN])
        xeoh[:, :, F_IN:] = oh_core
        per_core.append((idx_pb, xeoh, oh_core))
    return per_core


def kernel(x, edge_attr, W1, b1, W2, b2, edge_index, batch_size, seq_len,
           n_nodes):
    x = np.asarray(x, np.float32)
    edge_attr = np.asarray(edge_attr, np.float32)
    W1 = np.asarray(W1, np.float32)
    b1 = np.asarray(b1, np.float32)
    W2 = np.asarray(W2, np.float32)
    b2 = np.asarray(b2, np.float32)
    edge_index = np.asarray(edge_index)
    assert x.shape == (N, F_IN) and edge_index.shape == (2, E)

    per_core = _host_prep(x, edge_attr, edge_index)

    # ---- launch 1 ----
    if "l1" not in _NC_CACHE:
        _NC_CACHE["l1"] = _build_l1()
    nc1 = _NC_CACHE["l1"]

    w1_in = np.ascontiguousarray(W1.transpose(1, 0, 2)).astype(np.float16)
    b1_in = np.ascontiguousarray(
        b1.reshape(NT, 2, 128).transpose(2, 0, 1).reshape(128, NT * 2))
    w2_in = np.ascontiguousarray(
        W2.reshape(NT, 2, 128, D2).transpose(2, 0, 1, 3)).astype(np.float16)

    in_maps1 = []
    for k in range(NCORES):
        _, xeoh, _ = per_core[k]
        in_maps1.append({
            "xeoh": xeoh, "w1": w1_in, "b1": b1_in, "w2": w2_in,
        })
    res1 = run_bass_kernel_spmd(
        nc1, in_maps1, core_ids=list(range(NCORES)), trace=TRACE)
    if TRACE:
        LAST_TIMING["l1_ns"] = res1.exec_time_ns

    g_full = np.concatenate(
        [res1.results[k]["g16"] for k in range(NCORES)], axis=0)  # [N, 384] f16

    # ---- launch 2 ----
    if "l2" not in _NC_CACHE:
        _NC_CACHE["l2"] = _build_l2()
    nc2 = _NC_CACHE["l2"]

    GW = NT * D2
    b2_in = np.ascontiguousarray(b2.T)                            # [128, 3]
    in_maps2 = []
    for k in range(NCORES):
        idx_pb, _, oh_core = per_core[k]
        geoh = np.empty((128, BATCHES_PC, GW + W_OH), np.float16)
        np.take(g_full, idx_pb, axis=0, out=geoh[:, :, :GW])
        geoh[:, :, GW:] = oh_core
        in_maps2.append({"geoh": geoh, "b2": b2_in})
    res2 = run_bass_kernel_spmd(
        nc2, in_maps2, core_ids=list(range(NCORES)), trace=TRACE)
    if TRACE:
        LAST_TIMING["l2_ns"] = res2.exec_time_ns

    # per-core out2 [D2, TILES, NT, 128] -> [NT, D2, NPC]; concat cores
    m2t = np.concatenate(
        [res2.results[k]["out2"].transpose(2, 0, 1, 3).reshape(NT, D2, NPC)
         for k in range(NCORES)], axis=2)                          # [3,128,N] f16

    # [3, 128, (b, s, nn)] -> out[(b, nn), s, (t, d)]
    out = m2t.astype(np.float32).reshape(NT, D2, BATCH, SEQ, NNODE)
    out = out.transpose(2, 4, 3, 0, 1)
    out = np.ascontiguousarray(
        out.reshape(BATCH * NNODE, SEQ, NT * D2), dtype=np.float32)
    return out
